# revision 1
# baseline (speedup 1.0000x reference)
"""
Trainium2 Bass kernel for nn_MultiHeadAttention_74586402062628.

Data-parallel across 8 NeuronCores: one batch element per core.

Per-core formulation (B=8, S=1000, E=1024, H=16, D=64):
  - x [S,E] is transposed on-chip (PE transpose) to xT [E,S].
  - Q,K projections produce qT,kT [H*D, S] (head-dim on partitions; head h
    lives in 128-row tile h//2 at partition base (h%2)*64).
  - V projection produces v in natural layout [S, H*D], scattered per-head.
  - Attention is computed transposed: scoresT[t,s] = k_t . q_s / sqrt(S).
    exp() without max-subtraction (logits are tiny for this problem), causal
    mask applied as a 0/1 multiply with a sliding mask, and the softmax
    denominator is produced for free as an extra "ones" column in the AV
    matmul: psum = [v | 1]^T @ expT -> rows 0..63 = unnormalized zT, row 64
    (even heads) / 63 (odd heads, via tile_position col offset) = denom.
  - zT/denom -> yT [E, S] which directly feeds the output projection as the
    stationary operand; out[s,f] psum gets + bp via a K=1 ones matmul, then
    exact GELU on ScalarE.
  - bv is folded into an effective output bias bpe = bp + bv @ wp (valid
    because softmax rows sum to 1); bq/bk are added on PSUM->SBUF eviction.

Matmuls run as float32r (full PE rate at N>=256) with fp32 storage/PSUM.
"""

import math
import os
import sys

for _p in ("/opt/trn_rl_repo", "/opt/pypackages"):
    if _p not in sys.path:
        sys.path.insert(0, _p)

import numpy as np

B, S, E, H, D = 8, 1000, 1024, 16, 64
P = 128
NB = 8                      # 128-row blocks covering S (last is partial)
LAST = S - (NB - 1) * P     # 104
KT = 8                      # 128-row contraction tiles covering E
ST = ((0, 512), (512, 488))     # s tiles (start, width) covering S
FT = ((0, 512), (512, 512))     # f/n tiles covering E
SCALE = 1.0 / math.sqrt(S)
NCORES = 8

# matmul dtype: float32r = reduced-precision fp32 (full PE rate).
# Set BASSMHA_FULL_FP32=1 to fall back to exact (4x slower) fp32 matmuls.
_FULL_FP32 = os.environ.get("BASSMHA_FULL_FP32", "0") == "1"
# BASSMHA_NO_GELU=1: replace final GELU with Identity (CoreSim lacks Gelu)
_NO_GELU = os.environ.get("BASSMHA_NO_GELU", "0") == "1"

_CACHE = {}


def _build_nc():
    from concourse import bass, bacc
    import concourse.mybir as mybir
    from concourse import tile
    from concourse.masks import make_identity

    dt = mybir.dt
    f32 = dt.float32
    mdt = dt.float32 if _FULL_FP32 else dt.float32r
    AF = mybir.ActivationFunctionType
    Alu = mybir.AluOpType

    def mm(ap):
        return ap

    nc = bacc.Bacc("TRN2", debug=False, target_bir_lowering=False,
                   num_devices=NCORES)

    x_d = nc.declare_dram_parameter("x", [S, E], f32, isOutput=False)
    wq_d = nc.declare_dram_parameter("wq2", [E, E], f32, isOutput=False)
    wk_d = nc.declare_dram_parameter("wk2", [E, E], f32, isOutput=False)
    wv_d = nc.declare_dram_parameter("wv2", [E, E], f32, isOutput=False)
    wp_d = nc.declare_dram_parameter("wp2", [E, E], f32, isOutput=False)
    bq_d = nc.declare_dram_parameter("bqt", [P, KT], f32, isOutput=False)
    bk_d = nc.declare_dram_parameter("bkt", [P, KT], f32, isOutput=False)
    bp_d = nc.declare_dram_parameter("bpe", [1, E], f32, isOutput=False)
    out_d = nc.declare_dram_parameter("out", [S, E], f32, isOutput=True)

    MSKW = 896  # sliding causal mask width: diag offsets 0..384 + 512 cols

    with tile.TileContext(nc) as tc:
        with (
            tc.tile_pool(name="const", bufs=1) as constp,
            tc.tile_pool(name="persist", bufs=1) as persist,
        ):
            ident = constp.tile([P, P], f32)  # transpose runs in plain fp32
            make_identity(nc, ident[:])
            tri_msk = constp.tile([P, MSKW], f32)
            # msk[i, c] = 1.0 iff c - i - 384 >= 0; slice [:, 384-ld:...]
            # gives the causal keep-mask for a diag block at column offset ld.
            nc.gpsimd.memset(tri_msk[:], 1.0)
            nc.gpsimd.affine_select(
                out=tri_msk[:], in_=tri_msk[:],
                compare_op=Alu.is_ge, fill=0.0,
                base=-384, channel_multiplier=-1, pattern=[[1, MSKW]],
            )
            # column of ones on every partition: row r gives a [1, M]
            # ones lhsT at partition r for K=1 broadcast matmuls
            ones_full = constp.tile([P, P], mdt)
            nc.gpsimd.memset(ones_full[:].bitcast(f32), 1.0)

            bq_sb = constp.tile([P, KT], f32)
            nc.sync.dma_start(bq_sb[:], bq_d[:, :])
            bk_sb = constp.tile([P, KT], f32)
            nc.sync.dma_start(bk_sb[:], bk_d[:, :])
            bp_sb = constp.tile([1, E], mdt)
            nc.sync.dma_start(bp_sb[:], bp_d[:, :].bitcast(mdt))

            # Persistent activations
            qT = persist.tile([P, KT, S], mdt)       # [hd, m, s]
            kT = persist.tile([P, KT, S], mdt)
            # v stationary slabs per (pair, t_block), both parities:
            # [v(64) | ones] -> psum rows 0:64 = zT, row 64 = denom.
            # Odd-head z is later partition-shifted 0:64 -> 64:128 of yT
            # via a small SBUF->SBUF DMA (engines are partition-locked and
            # matmul col-offset output is rejected by this walrus).
            v_e = persist.tile([P, H // 2, NB, 65], mdt)
            v_o = persist.tile([P, H // 2, NB, 65], mdt)
            yT = persist.tile([P, KT, S], mdt)       # normalized z, stacked

            with tc.tile_pool(name="xT", bufs=1) as xtp:
                xT = xtp.tile([P, KT, S], mdt)

                # ---------------- Phase 1: transpose x -> xT ----------------
                with (
                    tc.tile_pool(name="xload", bufs=2) as xload,
                    tc.tile_pool(name="tpsum", bufs=4, space="PSUM") as tpsum,
                ):
                    for sb in range(NB):
                        rows = LAST if sb == NB - 1 else P
                        t0 = sb * P
                        xt = xload.tile([P, E], f32)
                        nc.sync.dma_start(xt[0:rows, :], x_d[t0:t0 + rows, :])
                        for eb in range(KT):
                            tp = tpsum.tile([P, P], f32)
                            nc.tensor.transpose(
                                tp[0:P, 0:rows],
                                xt[0:rows, eb * P:(eb + 1) * P],
                                ident[0:rows, 0:rows],
                            )
                            nc.scalar.activation(
                                xT[:, eb, t0:t0 + rows], tp[0:P, 0:rows],
                                AF.Copy)

                # ---------------- Phase 2: QKV projections ----------------
                # init v slabs: ones columns everywhere, then zero the
                # tb7 padding rows (96:128; the scatter copies rewrite rows
                # 0:104 afterwards, and rows 0:96 of tb<7 are fully written)
                nc.vector.memset(v_e[:, :, :, 64:65].bitcast(f32), 1.0)
                nc.vector.memset(v_o[:, :, :, 64:65].bitcast(f32), 1.0)
                nc.vector.memset(v_e[96:P, :, NB - 1, :].bitcast(f32), 0.0)
                nc.vector.memset(v_o[96:P, :, NB - 1, :].bitcast(f32), 0.0)

                with (
                    tc.tile_pool(name="wqk", bufs=12) as wqkp,
                    tc.tile_pool(name="wvp", bufs=8) as wvp,
                    tc.tile_pool(name="qkpsum", bufs=3, space="PSUM") as qkpsum,
                ):
                    # q and k: out tile m = heads 2m,2m+1; weight tiles
                    # loaded two m's at a time to halve DMA descriptor count.
                    # v(nt=0) is emitted right after m=0,1 so the first
                    # attention head-pairs can start while QKV continues.
                    def do_qk(wd, dst, bias, mg):
                        wts = []
                        for k in range(KT):
                            wt = wqkp.tile([P, 2 * P], mdt, tag="wqk")
                            nc.sync.dma_start(
                                wt[:],
                                wd[k * P:(k + 1) * P,
                                   mg * 2 * P:(mg + 1) * 2 * P].bitcast(mdt))
                            wts.append(wt)
                        for mi in range(2):
                            m = 2 * mg + mi
                            for (s0, W) in ST:
                                ps = qkpsum.tile([P, 512], f32, tag="ps",
                                                 name=f"qk_ps_{m}_{s0}")
                                for k in range(KT):
                                    nc.tensor.matmul(
                                        ps[0:P, 0:W],
                                        mm(wts[k][:, mi * P:(mi + 1) * P]),
                                        mm(xT[:, k, s0:s0 + W]),
                                        start=(k == 0), stop=(k == KT - 1),
                                    )
                                nc.vector.tensor_scalar_add(
                                    dst[:, m, s0:s0 + W], ps[0:P, 0:W],
                                    bias[:, m:m + 1])

                    def do_v(nt):
                        n0, Wn = FT[nt]
                        wvs = []
                        for k in range(KT):
                            wv = wvp.tile([P, 512], mdt, tag="wv")
                            nc.sync.dma_start(
                                wv[0:P, 0:Wn],
                                wv_d[k * P:(k + 1) * P,
                                     n0:n0 + Wn].bitcast(mdt))
                            wvs.append(wv)
                        for tb in range(NB):
                            rows = LAST if tb == NB - 1 else P
                            t0 = tb * P
                            ps = qkpsum.tile([P, 512], f32, tag="ps",
                                             name=f"v_ps_{nt}_{tb}")
                            for k in range(KT):
                                nc.tensor.matmul(
                                    ps[0:rows, 0:Wn],
                                    mm(xT[:, k, t0:t0 + rows]),
                                    mm(wvs[k][0:P, 0:Wn]),
                                    start=(k == 0), stop=(k == KT - 1),
                                )
                            src = ps[0:rows, 0:Wn].rearrange(
                                "p (h e) -> p h e", e=P)
                            hp0 = 4 * nt
                            nc.scalar.activation(
                                v_e[0:rows, hp0:hp0 + 4, tb, 0:64],
                                src[:, :, 0:64], AF.Copy)
                            nc.scalar.activation(
                                v_o[0:rows, hp0:hp0 + 4, tb, 0:64],
                                src[:, :, 64:128], AF.Copy)

                    do_qk(wq_d, qT, bq_sb, 0)
                    do_qk(wk_d, kT, bk_sb, 0)
                    do_v(0)
                    for mg in range(1, KT // 2):
                        do_qk(wq_d, qT, bq_sb, mg)
                        do_qk(wk_d, kT, bk_sb, mg)
                    do_v(1)

            # ---------------- Phase 3: attention ----------------
            # Both parities of a head pair share one 2-bank PSUM tile
            # ([*, 0:512] even / [*, 512:1024] odd) so exp / mask / recip /
            # copy run as single strided ops. zT matmuls trail the
            # scores+exp pipeline by PIPE steps so PE never waits on ACT.
            import dataclasses as _dc

            def g2(ap):
                return ap.rearrange("p (g c) -> p g c", g=2)

            def bcast2(ap):
                return _dc.replace(ap, ap=[ap.ap[0], [0, 2], ap.ap[1]])

            PIPE = 2
            with (
                tc.tile_pool(name="expp", bufs=5) as expp,
                tc.tile_pool(name="recipp", bufs=2) as recipp,
                tc.tile_pool(name="bcp", bufs=2) as bcp,
                tc.tile_pool(name="ztp", bufs=3) as ztp,
                tc.tile_pool(name="spsum", bufs=2, space="PSUM") as spsum,
                tc.tile_pool(name="zpsum", bufs=1, space="PSUM") as zpsum,
                tc.tile_pool(name="bcpsum", bufs=1, space="PSUM") as bcpsum,
            ):
                for hp in range(H // 2):
                    for (s0, W) in ST:
                        n_tb = (s0 + W + P - 1) // P
                        zp = zpsum.tile([P, 1024], f32, tag="zp")
                        exs = {}
                        geom = {}
                        for tb in range(n_tb):
                            rows = LAST if tb == NB - 1 else P
                            has_diag = tb * P >= s0
                            ld = tb * P - s0 if has_diag else 0
                            off = ld if (tb > 0 and ld > 0
                                         and W - ld >= 256) else 0
                            geom[tb] = (rows, has_diag, ld, off, W - off)
                        for i in range(n_tb + PIPE):
                            if i < n_tb:
                                tb = i
                                rows, has_diag, ld, off, N = geom[tb]
                                ldl = ld - off
                                t0 = tb * P
                                sp = spsum.tile([P, 1024], f32, tag="sp")
                                for par in range(2):
                                    base = par * 64
                                    nc.tensor.matmul(
                                        sp[0:rows, 512 * par:512 * par + N],
                                        mm(kT[base:base + 64, hp,
                                              t0:t0 + rows]),
                                        mm(qT[base:base + 64, hp,
                                              s0 + off:s0 + W]),
                                        start=True, stop=True,
                                    )
                                ex = expp.tile([P, 1024], mdt, tag="ex")
                                exv, spv = g2(ex[:, :]), g2(sp[:, :])
                                if rows < P:
                                    nc.vector.memset(
                                        exv[96:P, :, 0:N].bitcast(f32), 0.0)
                                nc.scalar.activation(
                                    exv[0:rows, :, 0:N], spv[0:rows, :, 0:N],
                                    AF.Exp, scale=SCALE)
                                if has_diag:
                                    dw = min(P, N - ldl)
                                    nc.vector.tensor_tensor(
                                        exv[0:rows, :, 0:ldl + dw],
                                        exv[0:rows, :, 0:ldl + dw],
                                        bcast2(tri_msk[0:rows,
                                                       384 - ldl:384 + dw]),
                                        op=Alu.mult)
                                exs[tb] = ex
                            j = i - PIPE
                            if 0 <= j < n_tb:
                                rows, has_diag, ld, off, N = geom[j]
                                ex = exs.pop(j)
                                for par, vs in ((0, v_e), (1, v_o)):
                                    nc.tensor.matmul(
                                        zp[0:65,
                                           512 * par + off:512 * par + W],
                                        mm(vs[:, hp, j, 0:65]),
                                        mm(ex[0:P,
                                              512 * par:512 * par + N]),
                                        start=(j == 0),
                                        stop=(j == n_tb - 1),
                                        skip_group_check=True,
                                    )
                        # normalize: yT[h] = zT / denom. One strided recip
                        # covers both denom rows; K=1 ones matmuls broadcast
                        # the reciprocal rows across partitions 0:64; one
                        # strided DVE copy brings them to SBUF; odd heads
                        # are partition-shifted into yT via SBUF->SBUF DMA.
                        rp = recipp.tile([P, 1024], mdt, tag="rp")
                        zpv, rpv = g2(zp[:, :]), g2(rp[:, :])
                        with nc.allow_low_precision(
                                reason="1/denom rounds to fp32r for the "
                                "broadcast matmul; within tolerance"):
                            nc.vector.reciprocal(
                                rpv[64:65, :, 0:W], zpv[64:65, :, 0:W])
                        bc = bcpsum.tile([P, 1024], f32, tag="bcps")
                        for par in range(2):
                            nc.tensor.matmul(
                                bc[0:64, 512 * par:512 * par + W],
                                ones_full[64:65, 0:64],
                                rp[64:65, 512 * par:512 * par + W],
                                start=True, stop=True)
                        bcs = bcp.tile([64, 1024], f32, tag="bc")
                        nc.vector.tensor_copy(
                            g2(bcs[:, :])[0:64, :, 0:W],
                            g2(bc[:, :])[0:64, :, 0:W])
                        nc.vector.tensor_tensor(
                            yT[0:64, hp, s0:s0 + W], zp[0:64, 0:W],
                            bcs[0:64, 0:W], op=Alu.mult)
                        zt_o = ztp.tile([64, 512], mdt, tag="zt")
                        nc.vector.tensor_tensor(
                            zt_o[0:64, 0:W], zp[0:64, 512:512 + W],
                            bcs[0:64, 512:512 + W], op=Alu.mult)
                        nc.sync.dma_start(
                            yT[64:P, hp, s0:s0 + W], zt_o[0:64, 0:W])

            # ---------------- Phase 4: output projection + GELU ----------------
            with (
                tc.tile_pool(name="wpp", bufs=9) as wpp,
                tc.tile_pool(name="outp", bufs=4) as outp,
                tc.tile_pool(name="fpsum", bufs=2, space="PSUM") as fpsum,
            ):
                for (f0, Fw) in FT:
                    wps = []
                    for k in range(KT):
                        wp = wpp.tile([P, 512], mdt, tag="wp")
                        nc.sync.dma_start(
                            wp[0:P, 0:Fw],
                            wp_d[k * P:(k + 1) * P, f0:f0 + Fw].bitcast(mdt))
                        wps.append(wp)
                    for sb in range(NB):
                        rows = LAST if sb == NB - 1 else P
                        r0 = sb * P
                        ps = fpsum.tile([P, 512], f32)
                        for k in range(KT):
                            nc.tensor.matmul(
                                ps[0:rows, 0:Fw],
                                mm(yT[:, k, r0:r0 + rows]),
                                mm(wps[k][0:P, 0:Fw]),
                                start=(k == 0), stop=False,
                            )
                        # + bias row via K=1 ones matmul
                        nc.tensor.matmul(
                            ps[0:rows, 0:Fw],
                            mm(ones_full[0:1, 0:rows]),
                            mm(bp_sb[0:1, f0:f0 + Fw]),
                            start=False, stop=True,
                        )
                        ot = outp.tile([P, 512], f32, tag="ot")
                        act = AF.Identity if _NO_GELU else AF.Gelu
                        nc.scalar.activation(
                            ot[0:rows, 0:Fw], ps[0:rows, 0:Fw], act)
                        nc.sync.dma_start(
                            out_d[r0:r0 + rows, f0:f0 + Fw], ot[0:rows, 0:Fw])

    nc.compile()
    return nc


def get_nc():
    if "nc" not in _CACHE:
        _CACHE["nc"] = _build_nc()
    return _CACHE["nc"]


def make_in_maps(inputs):
    x = np.ascontiguousarray(np.asarray(inputs["x"], np.float32))
    wq = np.asarray(inputs["wq"], np.float32)
    wk = np.asarray(inputs["wk"], np.float32)
    wv = np.asarray(inputs["wv"], np.float32)
    wp = np.asarray(inputs["wp"], np.float32)
    bq = np.asarray(inputs["bq"], np.float32)
    bk = np.asarray(inputs["bk"], np.float32)
    bv = np.asarray(inputs["bv"], np.float32)
    bp = np.asarray(inputs["bp"], np.float32)

    # [H, E, D] -> [E, H*D] (concat head outputs along columns)
    wq2 = np.ascontiguousarray(wq.transpose(1, 0, 2).reshape(E, E))
    wk2 = np.ascontiguousarray(wk.transpose(1, 0, 2).reshape(E, E))
    wv2 = np.ascontiguousarray(wv.transpose(1, 0, 2).reshape(E, E))
    wp2 = np.ascontiguousarray(wp)
    # per-partition bias layout: bqt[p, m] = bq_flat[m*128 + p]
    bqt = np.ascontiguousarray(bq.reshape(-1).reshape(KT, P).T)
    bkt = np.ascontiguousarray(bk.reshape(-1).reshape(KT, P).T)
    # fold bv into output bias: y = z + bv  =>  out += bv @ wp
    bpe = (bp.astype(np.float64)
           + bv.reshape(-1).astype(np.float64) @ wp.astype(np.float64))
    bpe = np.ascontiguousarray(bpe.astype(np.float32).reshape(1, E))

    shared = {"wq2": wq2, "wk2": wk2, "wv2": wv2, "wp2": wp2,
              "bqt": bqt, "bkt": bkt, "bpe": bpe}
    return [dict(shared, x=np.ascontiguousarray(x[b])) for b in range(B)]


def run(inputs, trace=False):
    from concourse.bass_utils import run_bass_kernel_spmd
    nc = get_nc()
    in_maps = make_in_maps(inputs)
    res = run_bass_kernel_spmd(nc, in_maps, list(range(NCORES)), trace=trace)
    out = np.stack([np.asarray(res.results[i]["out"]) for i in range(NCORES)])
    return out.astype(np.float32), res


def kernel(**inputs):
    out, _ = run(inputs, trace=False)
    return out



# revision 5
# speedup vs baseline: 1.6516x; 1.6516x over previous
"""
Trainium2 Bass kernel for nn_MultiHeadAttention_74586402062628.

Data-parallel across 8 NeuronCores: one batch element per core.

Per-core formulation (B=8, S=1000, E=1024, H=16, D=64), bf16 matmuls:
  - x [S,E] bf16 is transposed on-chip (PE transpose) to xT [E,S] bf16.
  - Q,K projections produce qT,kT [H*D, S] bf16 (head-dim on partitions;
    head h lives in 128-row tile h//2 at partition base (h%2)*64).
  - V projection produces v in natural layout [S, H*D] bf16, scattered
    per-head into [v(64) | ones] slabs so the AV matmul's psum row 64
    yields the softmax denominator for free.
  - Attention is computed transposed: scoresT[t,s] = k_t . q_s / sqrt(S).
    bf16 matmuls run at 1 cyc/row at ANY free dim, so every scores/AV
    tile is trimmed to exactly the unmasked columns (off = t0-s0). The
    causal diagonal is applied with a gpsimd affine_select (keep c >= p)
    directly on the exp tile; exp() without max-subtraction (logits are
    tiny for this problem).
  - Normalize: zp psum is evicted early to SBUF (frees the accumulator
    for the next head pair), 1/denom via reciprocal_approx_fast (single
    DVE op), broadcast across partitions with a K=1 fp32r ones-matmul,
    then one multiply per parity; odd heads partition-shift into yT via
    SBUF->SBUF DMA.
  - Output projection streams resident wp bf16; bias row added via a
    K=1 ones matmul; exact GELU on ScalarE.
  - bv is folded into an effective output bias bpe = bp + bv @ wp (valid
    because softmax rows sum to 1); bq/bk are added on PSUM->SBUF
    eviction (fp32).
"""

import math
import os
import sys

for _p in ("/opt/trn_rl_repo", "/opt/pypackages"):
    if _p not in sys.path:
        sys.path.insert(0, _p)

import numpy as np

B, S, E, H, D = 8, 1000, 1024, 16, 64
P = 128
NB = 8                      # 128-row blocks covering S (last is partial)
LAST = S - (NB - 1) * P     # 104
KT = 8                      # 128-row contraction tiles covering E
ST = ((0, 512), (512, 488))     # s tiles (start, width) covering S
FT = ((0, 512), (512, 512))     # f/n tiles covering E
SCALE = 1.0 / math.sqrt(S)
NCORES = 8

# BASSMHA_NO_GELU=1: replace final GELU with Identity (CoreSim lacks Gelu)
_NO_GELU = os.environ.get("BASSMHA_NO_GELU", "0") == "1"

_CACHE = {}


def _build_nc():
    from concourse import bass, bacc
    import concourse.mybir as mybir
    from concourse import tile
    from concourse.masks import make_identity

    dt = mybir.dt
    f32 = dt.float32
    f32r = dt.float32r
    bf16 = dt.bfloat16
    AF = mybir.ActivationFunctionType
    Alu = mybir.AluOpType

    nc = bacc.Bacc("TRN2", debug=False, target_bir_lowering=False,
                   num_devices=NCORES)

    x_d = nc.declare_dram_parameter("x", [S, E], bf16, isOutput=False)
    wq_d = nc.declare_dram_parameter("wq2", [E, E], bf16, isOutput=False)
    wk_d = nc.declare_dram_parameter("wk2", [E, E], bf16, isOutput=False)
    wv_d = nc.declare_dram_parameter("wv2", [E, E], bf16, isOutput=False)
    wp_d = nc.declare_dram_parameter("wp2", [E, E], bf16, isOutput=False)
    bq_d = nc.declare_dram_parameter("bqt", [P, KT], f32, isOutput=False)
    bk_d = nc.declare_dram_parameter("bkt", [P, KT], f32, isOutput=False)
    bp_d = nc.declare_dram_parameter("bpe", [1, E], bf16, isOutput=False)
    out_d = nc.declare_dram_parameter("out", [S, E], f32, isOutput=True)

    def g2(ap):
        return ap.rearrange("p (g c) -> p g c", g=2)

    with tile.TileContext(nc) as tc:
        with (
            tc.tile_pool(name="const", bufs=1) as constp,
            tc.tile_pool(name="persist", bufs=1) as persist,
        ):
            ident = constp.tile([P, P], bf16)
            make_identity(nc, ident[:])
            # ones on every partition, f32r for the K=1 recip broadcast
            ones_r = constp.tile([P, P], f32r)
            nc.gpsimd.memset(ones_r[:].bitcast(f32), 1.0)
            # single ones row (partition 0) for the K=1 output-bias matmul
            ones_b = constp.tile([1, P], bf16)
            nc.gpsimd.memset(ones_b[:], 1.0)

            bq_sb = constp.tile([P, KT], f32)
            nc.sync.dma_start(bq_sb[:], bq_d[:, :])
            bk_sb = constp.tile([P, KT], f32)
            nc.sync.dma_start(bk_sb[:], bk_d[:, :])
            bp_sb = constp.tile([1, E], bf16)
            nc.sync.dma_start(bp_sb[:], bp_d[:, :])

            # Persistent activations
            qT = persist.tile([P, KT, S], bf16)      # [hd, m, s]
            kT = persist.tile([P, KT, S], bf16)
            # v stationary slabs per (pair, t_block), both parities:
            # [v(64) | ones] -> psum rows 0:64 = zT, row 64 = denom.
            v_e = persist.tile([P, H // 2, NB, 65], bf16)
            v_o = persist.tile([P, H // 2, NB, 65], bf16)
            yT = persist.tile([P, KT, S], bf16)      # normalized z, stacked

            with tc.tile_pool(name="xT", bufs=1) as xtp:
                xT = xtp.tile([P, KT, S], bf16)

                # ---------------- Phase 1: transpose x -> xT ----------------
                with (
                    tc.tile_pool(name="xload", bufs=2) as xload,
                    tc.tile_pool(name="tpsum", bufs=4, space="PSUM") as tpsum,
                ):
                    for sb in range(NB):
                        rows = LAST if sb == NB - 1 else P
                        t0 = sb * P
                        xt = xload.tile([P, E], bf16)
                        nc.sync.dma_start(xt[0:rows, :], x_d[t0:t0 + rows, :])
                        for eb in range(KT):
                            tp = tpsum.tile([P, P], bf16)
                            nc.tensor.transpose(
                                tp[0:P, 0:rows],
                                xt[0:rows, eb * P:(eb + 1) * P],
                                ident[0:rows, 0:rows],
                            )
                            nc.scalar.activation(
                                xT[:, eb, t0:t0 + rows], tp[0:P, 0:rows],
                                AF.Copy)

                # ---------------- Phase 2: QKV projections ----------------
                # init v slabs: ones columns everywhere, then zero the
                # tb7 padding rows (96:128; the scatter copies rewrite rows
                # 0:104 afterwards, and rows 0:96 of tb<7 are fully written)
                nc.gpsimd.memset(v_e[:, :, :, 64:65], 1.0)
                nc.gpsimd.memset(v_o[:, :, :, 64:65], 1.0)
                nc.gpsimd.memset(v_e[96:P, :, NB - 1, :], 0.0)
                nc.gpsimd.memset(v_o[96:P, :, NB - 1, :], 0.0)

                with (
                    tc.tile_pool(name="wqkv", bufs=1) as wqkvp,
                    tc.tile_pool(name="qkpsum", bufs=3, space="PSUM") as qkpsum,
                ):
                    wq_sb = wqkvp.tile([P, KT, E], bf16)
                    wk_sb = wqkvp.tile([P, KT, E], bf16)
                    wv_sb = wqkvp.tile([P, KT, E], bf16)
                    for k in range(KT):
                        nc.sync.dma_start(wq_sb[:, k, :],
                                          wq_d[k * P:(k + 1) * P, :])
                        nc.sync.dma_start(wk_sb[:, k, :],
                                          wk_d[k * P:(k + 1) * P, :])
                        nc.sync.dma_start(wv_sb[:, k, :],
                                          wv_d[k * P:(k + 1) * P, :])

                    for m in range(KT):
                        for w_sb, dst, bias in ((wq_sb, qT, bq_sb),
                                                (wk_sb, kT, bk_sb)):
                            for (s0, W) in ST:
                                ps = qkpsum.tile([P, 512], f32, tag="ps",
                                                 name=f"qk_ps_{m}_{s0}")
                                for k in range(KT):
                                    nc.tensor.matmul(
                                        ps[0:P, 0:W],
                                        w_sb[:, k, m * P:(m + 1) * P],
                                        xT[:, k, s0:s0 + W],
                                        start=(k == 0), stop=(k == KT - 1),
                                    )
                                nc.vector.tensor_scalar_add(
                                    dst[:, m, s0:s0 + W], ps[0:P, 0:W],
                                    bias[:, m:m + 1])

                    for nt in range(2):
                        n0, Wn = FT[nt]
                        for tb in range(NB):
                            rows = LAST if tb == NB - 1 else P
                            t0 = tb * P
                            ps = qkpsum.tile([P, 512], f32, tag="ps",
                                             name=f"v_ps_{nt}_{tb}")
                            for k in range(KT):
                                nc.tensor.matmul(
                                    ps[0:rows, 0:Wn],
                                    xT[:, k, t0:t0 + rows],
                                    wv_sb[:, k, n0:n0 + Wn],
                                    start=(k == 0), stop=(k == KT - 1),
                                )
                            src = ps[0:rows, 0:Wn].rearrange(
                                "p (h e) -> p h e", e=P)
                            hp0 = 4 * nt
                            nc.scalar.activation(
                                v_e[0:rows, hp0:hp0 + 4, tb, 0:64],
                                src[:, :, 0:64], AF.Copy)
                            nc.scalar.activation(
                                v_o[0:rows, hp0:hp0 + 4, tb, 0:64],
                                src[:, :, 64:128], AF.Copy)

            # wp resident for the output projection; DMA overlaps attention
            with tc.tile_pool(name="wp", bufs=1) as wpp:
                wp_sb = wpp.tile([P, KT, E], bf16)
                for k in range(KT):
                    nc.sync.dma_start(wp_sb[:, k, :],
                                      wp_d[k * P:(k + 1) * P, :])

                # ---------------- Phase 3: attention ----------------
                # Transposed-scores pipeline; zT matmuls trail the scores+exp
                # pipeline by PIPE steps so PE never waits on ACT.
                PIPE = 2
                with (
                    tc.tile_pool(name="expp", bufs=5) as expp,
                    tc.tile_pool(name="zsbp", bufs=2) as zsbp,
                    tc.tile_pool(name="rcpp", bufs=2) as rcpp,
                    tc.tile_pool(name="ztop", bufs=2) as ztop,
                    tc.tile_pool(name="spsum", bufs=2, space="PSUM") as spsum,
                    tc.tile_pool(name="zpsum", bufs=1, space="PSUM") as zpsum,
                    tc.tile_pool(name="bcpsum", bufs=1, space="PSUM") as bcpsum,
                ):
                    for (s0, W) in ST:
                        n_tb = (s0 + W + P - 1) // P
                        for hp in range(H // 2):
                            zp = zpsum.tile([P, 1024], f32, tag="zp")
                            exs = {}
                            geom = {}
                            for tb in range(n_tb):
                                rows = LAST if tb == NB - 1 else P
                                t0 = tb * P
                                off = t0 - s0 if t0 >= s0 else 0
                                geom[tb] = (rows, t0 >= s0, off, W - off)
                            for i in range(n_tb + PIPE):
                                if i < n_tb:
                                    tb = i
                                    rows, has_diag, off, N = geom[tb]
                                    t0 = tb * P
                                    sp = spsum.tile([P, 1024], f32, tag="sp")
                                    for par in range(2):
                                        base = par * 64
                                        nc.tensor.matmul(
                                            sp[0:rows,
                                               512 * par:512 * par + N],
                                            kT[base:base + 64, hp,
                                               t0:t0 + rows],
                                            qT[base:base + 64, hp,
                                               s0 + off:s0 + W],
                                            start=True, stop=True,
                                        )
                                    ex = expp.tile([P, 1024], bf16, tag="ex")
                                    exv, spv = g2(ex[:, :]), g2(sp[:, :])
                                    if rows < P:
                                        nc.vector.memset(
                                            exv[96:P, :, 0:N], 0.0)
                                    nc.scalar.activation(
                                        exv[0:rows, :, 0:N],
                                        spv[0:rows, :, 0:N],
                                        AF.Exp, scale=SCALE)
                                    if has_diag:
                                        dw = min(rows, N)
                                        nc.gpsimd.affine_select(
                                            out=exv[0:rows, :, 0:dw],
                                            in_=exv[0:rows, :, 0:dw],
                                            compare_op=Alu.is_ge, fill=0.0,
                                            base=0, channel_multiplier=-1,
                                            pattern=[[0, 2], [1, dw]],
                                        )
                                    exs[tb] = ex
                                j = i - PIPE
                                if 0 <= j < n_tb:
                                    rows, has_diag, off, N = geom[j]
                                    ex = exs.pop(j)
                                    for par, vs in ((0, v_e), (1, v_o)):
                                        nc.tensor.matmul(
                                            zp[0:65,
                                               512 * par + off:512 * par + W],
                                            vs[:, hp, j, 0:65],
                                            ex[0:P,
                                               512 * par:512 * par + N],
                                            start=(j == 0),
                                            stop=(j == n_tb - 1),
                                            skip_group_check=True,
                                        )
                            # normalize: evict zp to SBUF first (frees the
                            # accumulator; f32r so the denom row can feed the
                            # f32r ones-matmul), broadcast the denominator
                            # across partitions 0:64, 1/denom via the fast
                            # approx on the broadcast, multiply on GpSimd.
                            zsb = zsbp.tile([65, 1024], f32r, tag="zsb")
                            with nc.allow_low_precision(
                                    reason="z/denom round to fp32r for the "
                                    "broadcast matmul; within tolerance"):
                                nc.vector.tensor_copy(
                                    g2(zsb[:, :])[0:65, :, 0:W],
                                    g2(zp[:, :])[0:65, :, 0:W])
                            bc = bcpsum.tile([P, 1024], f32, tag="bcps")
                            for par in range(2):
                                nc.tensor.matmul(
                                    bc[0:64, 512 * par:512 * par + W],
                                    ones_r[64:65, 0:64],
                                    zsb[64:65, 512 * par:512 * par + W],
                                    start=True, stop=True)
                            rcp = rcpp.tile([64, 1024], f32, tag="rcp")
                            for par in range(2):
                                nc.vector.reciprocal_approx_fast(
                                    rcp[0:64, 512 * par:512 * par + W],
                                    bc[0:64, 512 * par:512 * par + W])
                            nc.gpsimd.tensor_tensor(
                                yT[0:64, hp, s0:s0 + W],
                                zsb[0:64, 0:W].bitcast(f32),
                                rcp[0:64, 0:W], op=Alu.mult)
                            zto = ztop.tile([64, 512], bf16, tag="zto")
                            nc.gpsimd.tensor_tensor(
                                zto[0:64, 0:W],
                                zsb[0:64, 512:512 + W].bitcast(f32),
                                rcp[0:64, 512:512 + W], op=Alu.mult)
                            nc.sync.dma_start(
                                yT[64:P, hp, s0:s0 + W], zto[0:64, 0:W])

                # ---------- Phase 4: output projection + GELU ----------
                with (
                    tc.tile_pool(name="outp", bufs=4) as outp,
                    tc.tile_pool(name="fpsum", bufs=2, space="PSUM") as fpsum,
                ):
                    for sb in range(NB):
                        rows = LAST if sb == NB - 1 else P
                        r0 = sb * P
                        for (f0, Fw) in FT:
                            ps = fpsum.tile([P, 512], f32)
                            for k in range(KT):
                                nc.tensor.matmul(
                                    ps[0:rows, 0:Fw],
                                    yT[:, k, r0:r0 + rows],
                                    wp_sb[:, k, f0:f0 + Fw],
                                    start=(k == 0), stop=False,
                                )
                            # + bias row via K=1 ones matmul
                            nc.tensor.matmul(
                                ps[0:rows, 0:Fw],
                                ones_b[0:1, 0:rows],
                                bp_sb[0:1, f0:f0 + Fw],
                                start=False, stop=True,
                            )
                            ot = outp.tile([P, 512], f32, tag="ot")
                            act = AF.Identity if _NO_GELU else AF.Gelu
                            nc.scalar.activation(
                                ot[0:rows, 0:Fw], ps[0:rows, 0:Fw], act)
                            nc.sync.dma_start(
                                out_d[r0:r0 + rows, f0:f0 + Fw],
                                ot[0:rows, 0:Fw])

    nc.compile()
    return nc


def get_nc():
    if "nc" not in _CACHE:
        _CACHE["nc"] = _build_nc()
    return _CACHE["nc"]


def make_in_maps(inputs):
    import ml_dtypes
    bf16 = ml_dtypes.bfloat16

    x = np.asarray(inputs["x"], np.float32)
    wq = np.asarray(inputs["wq"], np.float32)
    wk = np.asarray(inputs["wk"], np.float32)
    wv = np.asarray(inputs["wv"], np.float32)
    wp = np.asarray(inputs["wp"], np.float32)
    bq = np.asarray(inputs["bq"], np.float32)
    bk = np.asarray(inputs["bk"], np.float32)
    bv = np.asarray(inputs["bv"], np.float32)
    bp = np.asarray(inputs["bp"], np.float32)

    # [H, E, D] -> [E, H*D] (concat head outputs along columns)
    wq2 = np.ascontiguousarray(
        wq.transpose(1, 0, 2).reshape(E, E).astype(bf16))
    wk2 = np.ascontiguousarray(
        wk.transpose(1, 0, 2).reshape(E, E).astype(bf16))
    wv2 = np.ascontiguousarray(
        wv.transpose(1, 0, 2).reshape(E, E).astype(bf16))
    wp2 = np.ascontiguousarray(wp.astype(bf16))
    # per-partition bias layout: bqt[p, m] = bq_flat[m*128 + p]
    bqt = np.ascontiguousarray(bq.reshape(-1).reshape(KT, P).T)
    bkt = np.ascontiguousarray(bk.reshape(-1).reshape(KT, P).T)
    # fold bv into output bias: y = z + bv  =>  out += bv @ wp
    bpe = (bp.astype(np.float64)
           + bv.reshape(-1).astype(np.float64) @ wp.astype(np.float64))
    bpe = np.ascontiguousarray(
        bpe.astype(np.float32).astype(bf16).reshape(1, E))

    shared = {"wq2": wq2, "wk2": wk2, "wv2": wv2, "wp2": wp2,
              "bqt": bqt, "bkt": bkt, "bpe": bpe}
    return [dict(shared, x=np.ascontiguousarray(x[b].astype(bf16)))
            for b in range(B)]


def run(inputs, trace=False):
    from concourse.bass_utils import run_bass_kernel_spmd
    nc = get_nc()
    in_maps = make_in_maps(inputs)
    res = run_bass_kernel_spmd(nc, in_maps, list(range(NCORES)), trace=trace)
    out = np.stack([np.asarray(res.results[i]["out"]) for i in range(NCORES)])
    return out.astype(np.float32), res


def kernel(**inputs):
    out, _ = run(inputs, trace=False)
    return out


# revision 9
# speedup vs baseline: 1.9196x; 1.1623x over previous
"""
Trainium2 Bass kernel for nn_MultiHeadAttention_74586402062628.

Data-parallel across 8 NeuronCores: one batch element per core.

Per-core formulation (B=8, S=1000, E=1024, H=16, D=64), bf16 matmuls:
  - x [S,E] bf16 is transposed on-chip (PE transpose) to xT [E,S] bf16;
    the V projection runs per t-block right behind each block's
    transposes so the PE is dense from the start (HAM warm-up).
  - Q,K projections produce qT,kT [H*D, S] bf16 (head-dim on partitions;
    head h lives in 128-row tile h//2 at partition base (h%2)*64).
    Attention for the first s-tile of head pair m is emitted right after
    projection m so its exp() hides under the projection stretch.
  - V is scattered per-head into [v(64) | ones] slabs so the AV matmul's
    psum row 64 yields the softmax denominator for free.
  - Attention is computed transposed: scoresT[t,s] = k_t . q_s / sqrt(S).
    bf16 matmuls run at 1 cyc/row at ANY free dim, so every scores/AV
    tile is trimmed to exactly the unmasked columns (off = t0-s0). The
    causal diagonal is applied with a gpsimd affine_select (keep c >= p)
    directly on the exp tile; exp() without max-subtraction (logits are
    tiny for this problem).
  - Normalize: zp psum is evicted early to SBUF (frees the accumulator;
    fp32r so the denom row can feed the fp32r ones-matmul broadcast),
    1/denom via reciprocal_approx_fast on the broadcast, multiplies on
    GpSimd; odd heads partition-shift into yT via SBUF->SBUF DMA.
  - Output projection for s<512 is interleaved with second-s-tile
    attention (which is exp-bound on ScalarE); bias row added via a K=1
    ones matmul; exact GELU on ScalarE.
  - bv is folded into an effective output bias bpe = bp + bv @ wp (valid
    because softmax rows sum to 1); bq/bk are added on PSUM->SBUF
    eviction (fp32).
"""

import math
import os
import sys

for _p in ("/opt/trn_rl_repo", "/opt/pypackages"):
    if _p not in sys.path:
        sys.path.insert(0, _p)

import numpy as np

B, S, E, H, D = 8, 1000, 1024, 16, 64
P = 128
NB = 8                      # 128-row blocks covering S (last is partial)
LAST = S - (NB - 1) * P     # 104
KT = 8                      # 128-row contraction tiles covering E
ST = ((0, 512), (512, 488))     # s tiles (start, width) covering S
FT = ((0, 512), (512, 512))     # f/n tiles covering E
SCALE = 1.0 / math.sqrt(S)
NCORES = 8
PIPE = 2

# BASSMHA_NO_GELU=1: replace final GELU with Identity (CoreSim lacks Gelu)
_NO_GELU = os.environ.get("BASSMHA_NO_GELU", "0") == "1"

_CACHE = {}


def _build_nc():
    from concourse import bass, bacc
    import concourse.mybir as mybir
    from concourse import tile
    from concourse.masks import make_identity

    dt = mybir.dt
    f32 = dt.float32
    f32r = dt.float32r
    bf16 = dt.bfloat16
    AF = mybir.ActivationFunctionType
    Alu = mybir.AluOpType

    nc = bacc.Bacc("TRN2", debug=False, target_bir_lowering=False,
                   num_devices=NCORES)

    x_d = nc.declare_dram_parameter("x", [S, E], bf16, isOutput=False)
    wq_d = nc.declare_dram_parameter("wq2", [E, E], bf16, isOutput=False)
    wk_d = nc.declare_dram_parameter("wk2", [E, E], bf16, isOutput=False)
    wv_d = nc.declare_dram_parameter("wv2", [E, E], bf16, isOutput=False)
    wp_d = nc.declare_dram_parameter("wp2", [E, E], bf16, isOutput=False)
    bq_d = nc.declare_dram_parameter("bqt", [P, KT], f32, isOutput=False)
    bk_d = nc.declare_dram_parameter("bkt", [P, KT], f32, isOutput=False)
    bp_d = nc.declare_dram_parameter("bpe", [1, E], bf16, isOutput=False)
    out_d = nc.declare_dram_parameter("out", [S, E], f32, isOutput=True)

    def g2(ap):
        return ap.rearrange("p (g c) -> p g c", g=2)

    with tile.TileContext(nc) as tc:
        with (
            tc.tile_pool(name="const", bufs=1) as constp,
            tc.tile_pool(name="persist", bufs=1) as persist,
        ):
            ident = constp.tile([P, P], bf16)
            make_identity(nc, ident[:])
            # ones on every partition, f32r for the K=1 denom broadcast
            ones_r = constp.tile([P, P], f32r)
            nc.gpsimd.memset(ones_r[:].bitcast(f32), 1.0)
            # single ones row (partition 0) for the K=1 output-bias matmul
            ones_b = constp.tile([1, P], bf16)
            nc.gpsimd.memset(ones_b[:], 1.0)

            # small biases first on the sync DMA queue, then the weights
            bq_sb = constp.tile([P, KT], f32)
            nc.sync.dma_start(bq_sb[:], bq_d[:, :])
            bk_sb = constp.tile([P, KT], f32)
            nc.sync.dma_start(bk_sb[:], bk_d[:, :])
            bp_sb = constp.tile([1, E], bf16)
            nc.sync.dma_start(bp_sb[:], bp_d[:, :])

            # Persistent activations
            qT = persist.tile([P, KT, S], bf16)      # [hd, m, s]
            kT = persist.tile([P, KT, S], bf16)
            v_e = persist.tile([P, H // 2, NB, 65], bf16)
            v_o = persist.tile([P, H // 2, NB, 65], bf16)
            yT = persist.tile([P, KT, S], bf16)      # normalized z, stacked

            # weights resident in SBUF for the whole kernel
            wq_sb = persist.tile([P, KT, E], bf16)
            wk_sb = persist.tile([P, KT, E], bf16)
            wv_sb = persist.tile([P, KT, E], bf16)
            wp_sb = persist.tile([P, KT, E], bf16)
            for w_sb, w_d in ((wv_sb, wv_d), (wq_sb, wq_d), (wk_sb, wk_d),
                              (wp_sb, wp_d)):
                for k in range(KT):
                    nc.sync.dma_start(w_sb[:, k, :], w_d[k * P:(k + 1) * P, :])

            # init v slabs: ones columns everywhere, then zero the tb7
            # padding rows (96:128; scatter rewrites rows 0:104 afterwards)
            nc.gpsimd.memset(v_e[:, :, :, 64:65], 1.0)
            nc.gpsimd.memset(v_o[:, :, :, 64:65], 1.0)
            nc.gpsimd.memset(v_e[96:P, :, NB - 1, :], 0.0)
            nc.gpsimd.memset(v_o[96:P, :, NB - 1, :], 0.0)

            with (
                tc.tile_pool(name="xT", bufs=1) as xtp,
                tc.tile_pool(name="qkpsum", bufs=2, space="PSUM") as qkpsum,
            ):
                xT = xtp.tile([P, KT, S], bf16)

                # ---- Phase 1: transpose x -> xT, V projection per block ----
                with (
                    tc.tile_pool(name="xload", bufs=3) as xload,
                    tc.tile_pool(name="tpsum", bufs=4, space="PSUM") as tpsum,
                ):
                    for sb in range(NB):
                        rows = LAST if sb == NB - 1 else P
                        t0 = sb * P
                        xt = xload.tile([P, E], bf16)
                        # x loads ride the gpsimd queue so they stream in
                        # parallel with the weight DMAs on the sync queue
                        nc.gpsimd.dma_start(xt[0:rows, :],
                                            x_d[t0:t0 + rows, :])
                        for eb in range(KT):
                            tp = tpsum.tile([P, P], bf16)
                            nc.tensor.transpose(
                                tp[0:P, 0:rows],
                                xt[0:rows, eb * P:(eb + 1) * P],
                                ident[0:rows, 0:rows],
                            )
                            nc.scalar.activation(
                                xT[:, eb, t0:t0 + rows], tp[0:P, 0:rows],
                                AF.Copy)
                        # V projection for this t-block, both column halves
                        for nt in range(2):
                            n0, Wn = FT[nt]
                            ps = qkpsum.tile([P, 512], f32, tag="ps",
                                             name=f"v_ps_{nt}_{sb}")
                            for k in range(KT):
                                nc.tensor.matmul(
                                    ps[0:rows, 0:Wn],
                                    xT[:, k, t0:t0 + rows],
                                    wv_sb[:, k, n0:n0 + Wn],
                                    start=(k == 0), stop=(k == KT - 1),
                                )
                            src = ps[0:rows, 0:Wn].rearrange(
                                "p (h e) -> p h e", e=P)
                            hp0 = 4 * nt
                            nc.scalar.activation(
                                v_e[0:rows, hp0:hp0 + 4, sb, 0:64],
                                src[:, :, 0:64], AF.Copy)
                            nc.scalar.activation(
                                v_o[0:rows, hp0:hp0 + 4, sb, 0:64],
                                src[:, :, 64:128], AF.Copy)

                # ---- attention pools (live through phases 2-4) ----
                with (
                    tc.tile_pool(name="expp", bufs=5) as expp,
                    tc.tile_pool(name="zsbp", bufs=2) as zsbp,
                    tc.tile_pool(name="rcpp", bufs=2) as rcpp,
                    tc.tile_pool(name="ztop", bufs=2) as ztop,
                    tc.tile_pool(name="spsum", bufs=2, space="PSUM") as spsum,
                    tc.tile_pool(name="zpsum", bufs=1, space="PSUM") as zpsum,
                ):
                    def attn(s0, W, hp):
                        n_tb = (s0 + W + P - 1) // P
                        zp = zpsum.tile([P, 1024], f32, tag="zp")
                        exs = {}
                        geom = {}
                        for tb in range(n_tb):
                            rows = LAST if tb == NB - 1 else P
                            t0 = tb * P
                            off = t0 - s0 if t0 >= s0 else 0
                            geom[tb] = (rows, t0 >= s0, off, W - off)
                        for i in range(n_tb + PIPE):
                            if i < n_tb:
                                tb = i
                                rows, has_diag, off, N = geom[tb]
                                t0 = tb * P
                                sp = spsum.tile([P, 1024], f32, tag="sp")
                                for par in range(2):
                                    base = par * 64
                                    nc.tensor.matmul(
                                        sp[0:rows, 512 * par:512 * par + N],
                                        kT[base:base + 64, hp, t0:t0 + rows],
                                        qT[base:base + 64, hp,
                                           s0 + off:s0 + W],
                                        start=True, stop=True,
                                    )
                                ex = expp.tile([P, 1024], bf16, tag="ex")
                                exv, spv = g2(ex[:, :]), g2(sp[:, :])
                                if rows < P:
                                    nc.vector.memset(exv[96:P, :, 0:N], 0.0)
                                nc.scalar.activation(
                                    exv[0:rows, :, 0:N], spv[0:rows, :, 0:N],
                                    AF.Exp, scale=SCALE)
                                if has_diag:
                                    dw = min(rows, N)
                                    nc.gpsimd.affine_select(
                                        out=exv[0:rows, :, 0:dw],
                                        in_=exv[0:rows, :, 0:dw],
                                        compare_op=Alu.is_ge, fill=0.0,
                                        base=0, channel_multiplier=-1,
                                        pattern=[[0, 2], [1, dw]],
                                    )
                                exs[tb] = ex
                            j = i - PIPE
                            if 0 <= j < n_tb:
                                rows, has_diag, off, N = geom[j]
                                ex = exs.pop(j)
                                for par, vs in ((0, v_e), (1, v_o)):
                                    nc.tensor.matmul(
                                        zp[0:65,
                                           512 * par + off:512 * par + W],
                                        vs[:, hp, j, 0:65],
                                        ex[0:P, 512 * par:512 * par + N],
                                        start=(j == 0), stop=(j == n_tb - 1),
                                        skip_group_check=True,
                                    )
                        # normalize: evict zp to SBUF (frees the accumulator;
                        # f32r so the denom row feeds the f32r ones-matmul),
                        # broadcast denom to partitions 0:64, fast reciprocal
                        # on the broadcast, multiply on GpSimd.
                        zsb = zsbp.tile([65, 1024], f32r, tag="zsb")
                        with nc.allow_low_precision(
                                reason="z/denom round to fp32r for the "
                                "broadcast matmul; within tolerance"):
                            nc.vector.tensor_copy(
                                g2(zsb[:, :])[0:65, :, 0:W],
                                g2(zp[:, :])[0:65, :, 0:W])
                        # bc reuses zp's banks (ring WAR dep on the copy)
                        bc = zpsum.tile([P, 1024], f32, tag="zp")
                        for par in range(2):
                            nc.tensor.matmul(
                                bc[0:64, 512 * par:512 * par + W],
                                ones_r[64:65, 0:64],
                                zsb[64:65, 512 * par:512 * par + W],
                                start=True, stop=True)
                        rcp = rcpp.tile([64, 1024], f32, tag="rcp")
                        for par in range(2):
                            nc.vector.reciprocal_approx_fast(
                                rcp[0:64, 512 * par:512 * par + W],
                                bc[0:64, 512 * par:512 * par + W])
                        nc.gpsimd.tensor_tensor(
                            yT[0:64, hp, s0:s0 + W],
                            zsb[0:64, 0:W].bitcast(f32),
                            rcp[0:64, 0:W], op=Alu.mult)
                        zto = ztop.tile([64, 512], bf16, tag="zto")
                        nc.gpsimd.tensor_tensor(
                            zto[0:64, 0:W],
                            zsb[0:64, 512:512 + W].bitcast(f32),
                            rcp[0:64, 512:512 + W], op=Alu.mult)
                        nc.sync.dma_start(
                            yT[64:P, hp, s0:s0 + W], zto[0:64, 0:W])

                    # ---- Phase 2: Q,K projections + first-s-tile attention --
                    for m in range(KT):
                        for w_sb, dst, bias in ((wq_sb, qT, bq_sb),
                                                (wk_sb, kT, bk_sb)):
                            for (s0, W) in ST:
                                ps = qkpsum.tile([P, 512], f32, tag="ps",
                                                 name=f"qk_ps_{m}_{s0}")
                                for k in range(KT):
                                    nc.tensor.matmul(
                                        ps[0:P, 0:W],
                                        w_sb[:, k, m * P:(m + 1) * P],
                                        xT[:, k, s0:s0 + W],
                                        start=(k == 0), stop=(k == KT - 1),
                                    )
                                nc.vector.tensor_scalar_add(
                                    dst[:, m, s0:s0 + W], ps[0:P, 0:W],
                                    bias[:, m:m + 1])
                        attn(ST[0][0], ST[0][1], m)

                    # ---- Phases 3+4: second-s-tile attention interleaved
                    # with the output projection for s < 512 (its psum
                    # tiles reuse the qkpsum pool) ----
                    with tc.tile_pool(name="outp", bufs=4) as outp:
                        def oproj(sb):
                            rows = LAST if sb == NB - 1 else P
                            r0 = sb * P
                            for (f0, Fw) in FT:
                                ps = qkpsum.tile([P, 512], f32, tag="ps",
                                                 name=f"o_ps_{sb}_{f0}")
                                for k in range(KT):
                                    nc.tensor.matmul(
                                        ps[0:rows, 0:Fw],
                                        yT[:, k, r0:r0 + rows],
                                        wp_sb[:, k, f0:f0 + Fw],
                                        start=(k == 0), stop=False,
                                    )
                                nc.tensor.matmul(
                                    ps[0:rows, 0:Fw],
                                    ones_b[0:1, 0:rows],
                                    bp_sb[0:1, f0:f0 + Fw],
                                    start=False, stop=True,
                                )
                                ot = outp.tile([P, 512], f32, tag="ot")
                                act = AF.Identity if _NO_GELU else AF.Gelu
                                nc.scalar.activation(
                                    ot[0:rows, 0:Fw], ps[0:rows, 0:Fw], act)
                                nc.sync.dma_start(
                                    out_d[r0:r0 + rows, f0:f0 + Fw],
                                    ot[0:rows, 0:Fw])

                        for hp in range(H // 2):
                            attn(ST[1][0], ST[1][1], hp)
                            if hp >= 4:
                                oproj(hp - 4)
                        for sb in range(4, NB):
                            oproj(sb)

    nc.compile()
    return nc


def get_nc():
    if "nc" not in _CACHE:
        _CACHE["nc"] = _build_nc()
    return _CACHE["nc"]


def make_in_maps(inputs):
    import ml_dtypes
    bf16 = ml_dtypes.bfloat16

    x = np.asarray(inputs["x"], np.float32)
    wq = np.asarray(inputs["wq"], np.float32)
    wk = np.asarray(inputs["wk"], np.float32)
    wv = np.asarray(inputs["wv"], np.float32)
    wp = np.asarray(inputs["wp"], np.float32)
    bq = np.asarray(inputs["bq"], np.float32)
    bk = np.asarray(inputs["bk"], np.float32)
    bv = np.asarray(inputs["bv"], np.float32)
    bp = np.asarray(inputs["bp"], np.float32)

    # [H, E, D] -> [E, H*D] (concat head outputs along columns)
    wq2 = np.ascontiguousarray(
        wq.transpose(1, 0, 2).reshape(E, E).astype(bf16))
    wk2 = np.ascontiguousarray(
        wk.transpose(1, 0, 2).reshape(E, E).astype(bf16))
    wv2 = np.ascontiguousarray(
        wv.transpose(1, 0, 2).reshape(E, E).astype(bf16))
    wp2 = np.ascontiguousarray(wp.astype(bf16))
    # per-partition bias layout: bqt[p, m] = bq_flat[m*128 + p]
    bqt = np.ascontiguousarray(bq.reshape(-1).reshape(KT, P).T)
    bkt = np.ascontiguousarray(bk.reshape(-1).reshape(KT, P).T)
    # fold bv into output bias: y = z + bv  =>  out += bv @ wp
    bpe = (bp.astype(np.float64)
           + bv.reshape(-1).astype(np.float64) @ wp.astype(np.float64))
    bpe = np.ascontiguousarray(
        bpe.astype(np.float32).astype(bf16).reshape(1, E))

    shared = {"wq2": wq2, "wk2": wk2, "wv2": wv2, "wp2": wp2,
              "bqt": bqt, "bkt": bkt, "bpe": bpe}
    return [dict(shared, x=np.ascontiguousarray(x[b].astype(bf16)))
            for b in range(B)]


def run(inputs, trace=False):
    from concourse.bass_utils import run_bass_kernel_spmd
    nc = get_nc()
    in_maps = make_in_maps(inputs)
    res = run_bass_kernel_spmd(nc, in_maps, list(range(NCORES)), trace=trace)
    out = np.stack([np.asarray(res.results[i]["out"]) for i in range(NCORES)])
    return out.astype(np.float32), res


def kernel(**inputs):
    out, _ = run(inputs, trace=False)
    return out
